# revision 92
# baseline (speedup 1.0000x reference)
"""Causal self-attention (B=4, T=2048, C=768, H=12) on 8 NeuronCores.

Sharding: core <-> (batch b = core//2, heads h0 = 6*(core%2) .. h0+5).
Each core computes its 6 heads' attention for its batch element plus the
partial output projection; the host sums the two half-head partials per batch.

Device algorithm (per core), shaped by the cost model (matmul cost =
out-free-size per <=128-contraction step, independent of partitions):
  1. Q^T,K^T via fp8(e4m3) DoubleRow matmuls (x and Wqk pre-quantized on
     the host with power-of-2 scales; 256-deep contraction per instruction,
     4x the fp16 rate).  The fp32 PSUM is evacuated to fp16 [d, T] tiles
     (2 heads per 128 partitions) with the descale and 1/sqrt(D) folded in.
     V stays fp16 (x@Wv errors hit y directly): natural [T, d] layout with
     a ones column per head (65-stride) for softmax denominators.
  2. Scores: S^T[tk, tq] = K^T.T @ Q^T per head (fp16, contraction d=64),
     one tk-block x both heads per 2-bank PSUM tile, exact-causal
     narrowing.  Exp on ACT into fp16 E tiles (this is the critical
     engine: ~116us); gpsimd affine_select masks the diagonal strips.
  3. PV flipped: O[tq, d] accumulated per 128-query block as
     out[128, 65] = E-block.T @ V_aug (lhsT = the E tiles as produced,
     full 128-contraction, 65-free) -- half the PE rows of the [65, tq]
     orientation.  The ones column yields denominators.
  4. Normalize on DVE (reciprocal + per-partition tensor_scalar_mul) into
     fp16 [tq, 128] tiles, PE-transposed (identity matmul) into
     OF[p] = O^T [128, T].
  5. Proj fp16: partial = OF.T @ W_proj; final projections borrow the dead
     S banks and the idle ACT for their PSUM evacuation.

Scheduling: topological emission; the Tile scheduler's priority dispatch
does the timing.  Score mms + exp carry an URGENT priority offset so ACT
(the bottleneck engine) never starves; each window's qk production runs
one window ahead; QKV / PV / proj fill the PE between score groups.  All
E tiles are banked in SBUF (window-local lifetime) so PV can lag freely.
PSUM: S 4 banks, O-window A/B 2, shared qkv/proj pool 2.
"""

import numpy as np

import concourse.bass as bass
import concourse.mybir as mybir
import concourse.tile as tile
from concourse import bacc
from concourse.bass_utils import run_bass_kernel_spmd

F32 = mybir.dt.float32
F16 = mybir.dt.float16
F8 = mybir.dt.float8e4

X8_SCALE = 4.0       # x quantization scale for fp8 QK production
W8_SCALE = 64.0      # wqk quantization scale
Q_SCALE = 0.125      # 1/sqrt(D), folded into the Q psum evacuation

T = 2048
C = 768
D = 64
HPC = 6          # heads per core
NCC = 6          # C / 128
NT = 16          # T / 128
NJ = 4           # T / 512
EXP = mybir.ActivationFunctionType.Exp




def _emit(nc, tc, xT, xT8, wqk, wv, wp, out):
    from contextlib import ExitStack
    with ExitStack() as ctx:
        pp = ctx.enter_context(tc.tile_pool(name="persist", bufs=1))

        qk = [pp.tile([128, T], F16, tag=f"qk{m}", name=f"qk{m}") for m in range(6)]
        vaug = [pp.tile([128, HPC * (D + 1)], F16, tag=f"v{t}", name=f"vaug{t}")
                for t in range(NT)]
        OF = [pp.tile([128, T], F16, tag=f"of{p}", name=f"of{p}") for p in range(3)]
        wp_t = pp.tile([128, 3 * C], F16, tag="wp", name="wp_t")

        # QKV-phase SBUF inputs (stay open; SBUF is plentiful)
        xt_t = [[pp.tile([128, 1024], F16, tag=f"xt{c}_{q}", name=f"xt{c}_{q}")
                 for q in range(2)] for c in range(NCC)]
        # fp8 copies for DoubleRow QK production: c-PAIR-major tiles [p,2,f]
        xt8_t = [[pp.tile([128, 2 * 1024], F8, tag=f"x8{cc}_{q}",
                          name=f"x8{cc}_{q}") for q in range(2)]
                 for cc in range(3)]
        # per-pair [Q-128|K-128] column slabs, c-major, fp8: one DMA per pair
        wqk_p = [pp.tile([128, NCC * 256], F8, tag=f"wqk{i}", name=f"wqkp{i}")
                 for i in range(3)]
        wv_t = pp.tile([128, NCC * HPC * D], F16, tag="wv", name="wv_t")

        epl = ctx.enter_context(tc.tile_pool(name="epool", bufs=40))
        onp = ctx.enter_context(tc.tile_pool(name="onorm", bufs=8))
        rcp = ctx.enter_context(tc.tile_pool(name="rcp", bufs=4))
        obp = ctx.enter_context(tc.tile_pool(name="obp", bufs=8))

        # PSUM: scores 4 banks, O windows 2 banks, shared mm (qkv+proj) 2
        sp = ctx.enter_context(tc.tile_pool(name="spsum", bufs=2, space="PSUM"))
        oA = ctx.enter_context(tc.tile_pool(name="opsumA", bufs=1, space="PSUM"))
        oB = ctx.enter_context(tc.tile_pool(name="opsumB", bufs=1, space="PSUM"))
        mmp = ctx.enter_context(tc.tile_pool(name="mmpool", bufs=2, space="PSUM"))

        warm = rcp.tile([1, 8], F32, tag="warm", name="warmup")
        nc.vector.memset(warm[:], 0.0)
        nc.scalar.activation(warm[0:1, 0:8], warm[0:1, 0:8], EXP)

        ident = pp.tile([128, 128], F16, tag="ident", name="ident")
        from concourse.masks import make_identity
        make_identity(nc, ident[:])

        # ---------------- input DMA (cold-start-ordered) -------------------
        nc.sync.dma_start(out=wqk_p[0].rearrange("p (c d) -> p c d", c=NCC),
                          in_=wqk[:, 0:256].rearrange("(c p) d -> p c d", p=128))
        for q4 in (0, 1):
            # first window's x8 in quarter pieces: j=0 scores start earlier
            for cc in range(3):
                nc.sync.dma_start(
                    out=xt8_t[cc][0].rearrange(
                        "p (two f) -> p two f", two=2)[:, :, 512 * q4:512 * (q4 + 1)],
                    in_=xT8[256 * cc:256 * (cc + 1),
                            512 * q4:512 * (q4 + 1)].rearrange(
                        "(two p) f -> p two f", p=128))
        for i in (1, 2):
            nc.sync.dma_start(
                out=wqk_p[i].rearrange("p (c d) -> p c d", c=NCC),
                in_=wqk[:, 256 * i:256 * (i + 1)].rearrange(
                    "(c p) d -> p c d", p=128))
        for c in range(NCC):
            nc.sync.dma_start(out=xt_t[c][0][:],
                              in_=xT[128 * c:128 * (c + 1), 0:1024])
        nc.sync.dma_start(out=wv_t.rearrange("p (c d) -> p c d", c=NCC),
                          in_=wv.rearrange("(c p) d -> p c d", p=128))
        for cc in range(3):
            nc.sync.dma_start(
                out=xt8_t[cc][1].rearrange("p (two f) -> p two f", two=2),
                in_=xT8[256 * cc:256 * (cc + 1), 1024:2048].rearrange(
                    "(two p) f -> p two f", p=128))
        for c in range(NCC):
            nc.sync.dma_start(out=xt_t[c][1][:],
                              in_=xT[128 * c:128 * (c + 1), 1024:2048])
        nc.sync.dma_start(out=wp_t.rearrange("p (c d) -> p c d", c=3),
                          in_=wp.rearrange("(c p) d -> p c d", p=128))

        # ---------------- unit emitters ------------------------------------
        def emit_qk(m, j):
            ps = mmp.tile([128, 512], F32, tag="mm", name=f"qkps{m}_{j}")
            pair, qcol = m % 3, 128 * (m // 3)
            wv8 = wqk_p[pair].rearrange("p (c two d) -> p c two d", two=2, d=256)
            for cc in range(3):
                x8 = xt8_t[cc][j // 2].rearrange("p (two f) -> p two f", two=2)
                nc.tensor.matmul(
                    out=ps[:],
                    lhsT=wv8[:, cc, :, qcol:qcol + 128],
                    rhs=x8[:, :, 512 * (j % 2):512 * (j % 2 + 1)],
                    start=(cc == 0), stop=(cc == 2),
                    perf_mode=mybir.MatmulPerfMode.DoubleRow,
                )
            # evacuate with the fp8 scale compensation (and 1/sqrt(D) for Q)
            sc = (Q_SCALE if m < 3 else 1.0) / (X8_SCALE * W8_SCALE)
            nc.vector.tensor_scalar_mul(qk[m][:, 512 * j:512 * (j + 1)],
                                        ps[:], sc)

        def emit_v(t):
            ps = mmp.tile([128, 512], F32, tag="mm", name=f"vps{t}")[:, 0:HPC * D]
            for c in range(NCC):
                nc.tensor.matmul(
                    out=ps[:],
                    lhsT=xt_t[c][t // 8][:, 128 * (t % 8):128 * (t % 8 + 1)],
                    rhs=wv_t[:, 384 * c:384 * (c + 1)],
                    start=(c == 0), stop=(c == NCC - 1),
                )
            vv = vaug[t].rearrange("p (h c) -> p h c", c=D + 1)
            nc.gpsimd.memset(vv[:, :, D:D + 1], 1.0)
            nc.vector.tensor_copy(vv[:, :, 0:D],
                                  ps.rearrange("p (h c) -> p h c", c=D))

        E = {}  # (p, j, i) -> E tile [128, 1024] = kblock i x [h0-512 | h1-512]

        def emit_scores(p, j, k):
            """Scores+exp for kblocks {2k, 2k+1} of window j, both heads."""
            QT, KT = qk[p], qk[3 + p]
            for i in (2 * k, 2 * k + 1):
                ss = sp.tile([128, 1024], F32, tag="s", name=f"s{p}{j}{i}")
                isl = slice(128 * i, 128 * (i + 1))
                off = max(0, 128 * i - 512 * j)
                for sub in (0, 1):
                    b0 = 64 * sub
                    nc.tensor.matmul(
                        out=ss[:, 512 * sub + off:512 * (sub + 1)],
                        lhsT=KT[b0:b0 + 64, isl],
                        rhs=QT[b0:b0 + 64, 512 * j + off:512 * (j + 1)],
                        start=True, stop=True,
                    )
                e = epl.tile([128, 1024], F16, tag="e", name=f"e{p}{j}{i}")
                if off <= 128:
                    # single call; for off=128 the 128 stale cols at the
                    # half boundary cost less than a second ACT init
                    nc.scalar.activation(e[:, off:1024], ss[:, off:1024], EXP)
                else:
                    nc.scalar.activation(e[:, off:512], ss[:, off:512], EXP)
                    nc.scalar.activation(e[:, 512 + off:1024],
                                         ss[:, 512 + off:1024], EXP)
                if i >= 4 * j:
                    for sub in (0, 1):
                        win = e[:, 512 * sub + off:512 * sub + off + 128]
                        nc.gpsimd.affine_select(
                            out=win, in_=win,
                            pattern=[[1, 128]],
                            compare_op=mybir.AluOpType.is_ge,
                            fill=0.0, base=0, channel_multiplier=-1,
                        )
                E[(p, j, i)] = e

        def emit_pv_qb(p, m, otile, qi):
            """O[128q, 65] accumulation for query block m into otile cols."""
            jw = m // 4
            for h in (0, 1):
                hh = 2 * p + h
                for kb in range(m + 1):
                    e = E[(p, jw, kb)]
                    qoff = 512 * h + 128 * m - 512 * jw
                    nc.tensor.matmul(
                        out=otile[:, 130 * qi + 65 * h:130 * qi + 65 * h + 65],
                        lhsT=e[:, qoff:qoff + 128],
                        rhs=vaug[kb][:, 65 * hh:65 * hh + 65],
                        start=(kb == 0), stop=(kb == m),
                    )

        def emit_norm(p, g, otile, qbs, tail=False):
            nq = len(qbs)
            rec = rcp.tile([128, 2 * nq], F32, tag="rec", name=f"rec{p}{g}")
            ov = otile.rearrange("p (q h c) -> p q h c", h=2, c=D + 1)
            nc.vector.reciprocal(
                rec.rearrange("p (q h c) -> p q h c", h=2, c=1),
                ov[:, :, :, D:D + 1])
            for qi, m in enumerate(qbs):
                onorm = onp.tile([128, 128], F16, tag="on", name=f"on{p}{m}")
                for h in (0, 1):
                    src = otile[:, 130 * qi + 65 * h:130 * qi + 65 * h + 64]
                    dst = onorm[:, 64 * h:64 * h + 64]
                    sc = rec[:, 2 * qi + h:2 * qi + h + 1]
                    nc.vector.tensor_scalar_mul(dst, src, sc)
                # PE transpose into O^T (keeps the chain off the DMA queues)
                tp = mmp.tile([128, 128], F16, tag="mm", padded_shape=[128, 1024],
                              name=f"tp{p}{m}")
                nc.tensor.transpose(tp[:], onorm[:], ident[:])
                nc.vector.tensor_copy(OF[p][:, 128 * m:128 * (m + 1)], tp[:])

        def emit_proj(t, tail=False):
            ob = obp.tile([128, C], F32, tag="ob", name=f"ob{t}")
            for half in (0, 1):
                if tail:
                    # the S banks are dead after the last exp: use them so
                    # the final proj skips the mm-pool rotation queue
                    ps = sp.tile([128, 1024], F32, tag="s",
                                 name=f"pj{t}_{half}")[:, 0:384]
                else:
                    ps = mmp.tile([128, 512], F32, tag="mm",
                                  name=f"pj{t}_{half}")[:, 0:384]
                for p3 in range(3):
                    nc.tensor.matmul(
                        out=ps[:],
                        lhsT=OF[p3][:, 128 * t:128 * (t + 1)],
                        rhs=wp_t[:, 768 * p3 + 384 * half:768 * p3 + 384 * (half + 1)],
                        start=(p3 == 0), stop=(p3 == 2),
                    )
                if tail:
                    nc.scalar.copy(ob[:, 384 * half:384 * (half + 1)], ps[:])
                else:
                    nc.vector.tensor_copy(ob[:, 384 * half:384 * (half + 1)],
                                          ps[:])
                # store each half as soon as its copy lands: overlaps the
                # transfer with the second half's evacuation
                nc.sync.dma_start(
                    out=out[128 * t:128 * (t + 1), 384 * half:384 * (half + 1)],
                    in_=ob[:, 384 * half:384 * (half + 1)])

        # ---------------- weaver -------------------------------------------

        # Topological emission; timing is left to the Tile scheduler's
        # ready-based priority dispatch.  Score mms + exp get maximal urgency
        # (priority offset) so PE always prefers a ready score group and ACT
        # never starves; QKV / PV / proj act as natural filler.
        # norm groups per window; window 3 ends in single-qblock groups to
        # shorten the final exp->norm->proj->store chain
        def wgroups(j):
            if j < 3:
                return [[4 * j, 4 * j + 1], [4 * j + 2, 4 * j + 3]]
            return [[12, 13], [14], [15]]

        opools = [oA, oB]
        ogi = 0
        norm_done = set()  # pair-2 qblocks with OF complete
        proj_emitted = set()

        def emit_pv_group(p, j, gi2, qbs):
            nonlocal ogi
            otile = opools[ogi % 2].tile([128, 130 * len(qbs)], F32,
                                         tag="o", name=f"o{p}{j}{gi2}")
            ogi += 1
            for qi, m in enumerate(qbs):
                emit_pv_qb(p, m, otile, qi)
            emit_norm(p, 2 * j + gi2, otile, qbs)
            if p == 2:
                norm_done.update(qbs)

        def emit_proj_ready(tmax=NT):
            for t in sorted(norm_done):
                if t not in proj_emitted and t < tmax:
                    proj_emitted.add(t)
                    emit_proj(t, tail=(t >= 12))

        URGENT = 1 << 20
        # prologue: window-0 inputs
        for m in (0, 3, 1, 4, 2, 5):
            emit_qk(m, 0)
        for j in range(NJ):
            for p in range(3):
                with tc.high_priority(offset=URGENT):
                    for k in range((4 * j + 4) // 2):
                        emit_scores(p, j, k)
            # produce the NEXT window's inputs while this window's exps run;
            # this window's v units (needed only by pv below) come after
            if j + 1 < NJ:
                for m in (0, 3, 1, 4, 2, 5):
                    emit_qk(m, j + 1)
            for t in range(4 * j, 4 * j + 4):
                emit_v(t)
            for p in range(3):
                for gi2, qbs in enumerate(wgroups(j)):
                    emit_pv_group(p, j, gi2, qbs)
            if j >= 1:
                emit_proj_ready()
        emit_proj_ready()


_NC_CACHE = None


def build_nc():
    global _NC_CACHE
    if _NC_CACHE is not None:
        return _NC_CACHE
    nc = bacc.Bacc(trn_type="TRN2")
    xT = nc.dram_tensor("xT", [C, T], F16, kind="ExternalInput").ap()
    xT8 = nc.dram_tensor("xT8", [C, T], F8, kind="ExternalInput").ap()
    wqk = nc.dram_tensor("wqk", [C, C], F8, kind="ExternalInput").ap()
    wv = nc.dram_tensor("wv", [C, HPC * D], F16, kind="ExternalInput").ap()
    wp = nc.dram_tensor("wp", [HPC * D, C], F16, kind="ExternalInput").ap()
    out = nc.dram_tensor("out", [T, C], F32, kind="ExternalOutput").ap()
    with tile.TileContext(nc) as tc:
        _emit(nc, tc, xT, xT8, wqk, wv, wp, out)
    nc.compile()
    _NC_CACHE = nc
    return nc


def make_in_maps(x, W_attn, W_proj):
    x = np.asarray(x, dtype=np.float32)
    W_attn = np.asarray(W_attn, dtype=np.float32)
    W_proj = np.asarray(W_proj, dtype=np.float32)
    in_maps = []
    import concourse.mybir as _mb
    f8 = _mb.dt.np(_mb.dt.float8e4)
    for core in range(8):
        b = core // 2
        h0 = HPC * (core % 2)
        xTf = np.ascontiguousarray(x[b].T)
        xT = xTf.astype(np.float16)
        xT8 = (xTf * np.float32(X8_SCALE)).astype(f8)
        q_cols = W_attn[:, 64 * h0:64 * h0 + 384]
        k_cols = W_attn[:, 768 + 64 * h0:768 + 64 * h0 + 384]
        # per-pair [Q-128|K-128] column slabs, fp8 with W8_SCALE
        wqk = (np.concatenate(
            [blk for i in range(3)
             for blk in (q_cols[:, 128 * i:128 * (i + 1)],
                         k_cols[:, 128 * i:128 * (i + 1)])],
            axis=1) * np.float32(W8_SCALE)).astype(f8)
        wv = W_attn[:, 1536 + 64 * h0:1536 + 64 * h0 + 384].astype(np.float16)
        wp = np.ascontiguousarray(
            W_proj[64 * h0:64 * h0 + 384, :]).astype(np.float16)
        in_maps.append({"xT": xT, "xT8": np.ascontiguousarray(xT8),
                        "wqk": np.ascontiguousarray(wqk),
                        "wv": np.ascontiguousarray(wv), "wp": wp})
    return in_maps


def kernel(x, W_attn, W_proj, _trace=False, _trace_kwargs=None):
    nc = build_nc()
    in_maps = make_in_maps(x, W_attn, W_proj)
    res = run_bass_kernel_spmd(nc, in_maps, list(range(8)), trace=_trace,
                               **(_trace_kwargs or {}))
    outs = [res.results[c]["out"] for c in range(8)]
    y = np.stack([outs[2 * b] + outs[2 * b + 1] for b in range(4)]).astype(np.float32)
    if _trace:
        return y, res
    return y
